# revision 33
# baseline (speedup 1.0000x reference)
"""Trainium2 Bass kernel for BiochemicalDynamics.

Reference computation (f32):
    Ax    = A @ x                                   # [N, DIM]
    s     = R * rowsum(x * Ax)                      # [N, 1]
    out   = F - B*x - s                             # [N, DIM]

Strategy: row-shard A across the 8 cores (1024 rows each) and compute
A @ x directly on the TensorEngine with A as the *moving* operand in
fp8 (e4m3) and x as the *stationary* operand, producing the transposed
product AxT = (A_loc @ x)^T in PSUM, accumulated over the 64 K-tiles of
the 8192-long contraction.  DoubleRow fp8 perf mode processes two
128-row K-tiles per matmul.

fp8 quantization error on A/x is zero-mean and averages out over the
8192-term reductions (~1.5e-3 relative on the output); the final
rowwise dot s_i = R * <x_i, (Ax)_i> uses a bf16 x, which kills the
error component that would NOT average (the x_i factor).  The output is
returned in bf16 (~2e-3 additional relative error; the harness gate is
2e-2).

Performance notes (from NTFF traces):
 - The A stream (8MB fp8/core) runs at the 16-engine DGE cap
   (~22-24 GB/s per engine, ~345 GB/s/core); everything else hides
   under it or sits in the fixed ~7us framework preamble / ~2us
   teardown windows.
 - DMA issue costs ~0.6us per DMA_DIRECT2D on the issuing engine, so A
   moves in 512KB super-tiles (4KB per-partition lines are the fastest
   observed packet size; 2KB is ~25% slower, 8KB gains nothing) on the
   Sync HWDGE queue, while x/consts/outputs ride the Scalar HWDGE
   queue.  (Alternating A tiles across both queues measured slightly
   worse than a pure single-queue A stream.)
 - The PE HAM clock-gate starts at half rate; ~4us of warmup matmuls
   bring it to full rate just as the first A tile lands.
 - The local 1024 output rows run as four column groups (512/256/128/
   128) streamed sequentially k-major, so groups 0-2 finish and run
   their epilogues while later groups still stream; only group 3's
   short [64, 128] chain (~1.6us: DVE stt -> reduce matmul -> Scalar
   copy -> store issue) sits in the tail.  Each group's reduce matmul
   is deferred past the next group's first super-tile so the in-order
   PE queue never stalls on the DVE.  Group 3 reuses group 0's PSUM
   banks (dead after group 0's epilogue; PSUM only has 8 banks).
   Epilogue math per group: tmp = R*xt (.) AxT (DVE), then two
   accumulating reduce-matmuls (W2 = [diag(-B); F-row] against
   [xt; ones], W1 = -1s against tmp), a PSUM->SBUF copy, the store.
 - Matmul PSUM writes must stay bank-aligned: a [64, 256] accumulation
   chain at a 1KB column offset inside a bank silently accumulates
   wrong results on TRN2 hardware (verified), so every chain owns a
   full [64, 512] bank.
"""

import sys

import numpy as np

for _p in ("/opt/trn_rl_repo", "/root/.axon_site/_ro/trn_rl_repo"):
    if _p not in sys.path:
        sys.path.append(_p)

N = 8192
DIM = 64
NCORES = 8
ROWS = N // NCORES       # 1024 rows of A per core
RH = 512                 # rows per i-half
F_CONST = 1.0
B_CONST = 0.1
R_CONST = 0.01

P = 128                  # SBUF partitions / K-tile size
KT = N // P              # 64 K-tiles in the contraction
KSUP = 8                 # K-tiles per DMA super-tile (512KB, 4KB lines)
NSUP = KT // KSUP        # 8 super-tiles per i-half
NWARM = 26               # PE warmup matmuls (~160ns each)

_CACHE = {}


def _build_nc():
    import concourse.mybir as mybir
    import concourse.tile as tile
    from concourse import bacc

    f32 = mybir.dt.float32
    bf16 = mybir.dt.bfloat16
    fp8 = mybir.dt.float8e4

    nc = bacc.Bacc(
        trn_type="TRN2", target_bir_lowering=False, debug=False, num_devices=NCORES
    )

    # a: A_loc^T packed per (half, super-tile): a[((h*8+st)*128+p), kk*512+i]
    #    = A[rows_{h*512+i}, ((st*8+kk)*128 + p)]   fp8, [2048, 4096].
    a = nc.dram_tensor("a", [2 * NSUP * P, KSUP * RH], fp8, kind="ExternalInput")
    # xp: x packed for stationary use: xp[p, kt, d] = x[kt*128+p, d], fp8.
    xp = nc.dram_tensor("xp", [P, KT, DIM], fp8, kind="ExternalInput")
    # xt: local x rows transposed, bf16 [64, 1024] (epilogue dot operand).
    xt = nc.dram_tensor("xt", [DIM, ROWS], bf16, kind="ExternalInput")
    # w2: epilogue reduce weights: rows 0..63 = diag(-B), row 64 = F.
    w2d = nc.dram_tensor("w2", [DIM + 1, DIM], bf16, kind="ExternalInput")
    out = nc.dram_tensor("out", [DIM, ROWS], bf16, kind="ExternalOutput")

    mult = mybir.AluOpType.mult
    DR = mybir.MatmulPerfMode.DoubleRow
    Copy = mybir.ActivationFunctionType.Copy

    with tile.TileContext(nc) as tc:
        with (
            tc.tile_pool(name="xpool", bufs=1) as xpool,
            tc.tile_pool(name="apool", bufs=16) as apool,
            tc.tile_pool(name="psum", bufs=1, space="PSUM") as psum_pool,
        ):
            # ---- Scalar-queue DMA: x (first K-tiles lead), consts.
            x_sb = xpool.tile([P, KT, DIM], fp8)
            nc.scalar.dma_start(out=x_sb[:, 0:KSUP, :], in_=xp[:, 0:KSUP, :])

            xv = xpool.tile([DIM + 1, ROWS], bf16, tag="xv")  # [xt; ones]
            w2 = xpool.tile([DIM + 1, DIM], bf16, tag="w2")
            nc.scalar.dma_start(out=xv[0:DIM, :], in_=xt[:])
            nc.scalar.dma_start(out=w2[:], in_=w2d[:])
            nc.scalar.dma_start(out=x_sb[:, KSUP:KT, :], in_=xp[:, KSUP:KT, :])
            nc.vector.memset(xv[DIM : DIM + 1, :], 1.0)

            w1 = xpool.tile([DIM, DIM], bf16, tag="w1")
            nc.vector.memset(w1[:], -1.0)
            tmp = xpool.tile([DIM, ROWS], bf16, tag="tmp")
            o_sb = xpool.tile([DIM, ROWS], bf16, tag="o")

            # ---- PE warmup: keeps the PE busy from the framework preamble
            # until the first A tile lands, lifting the HAM clock gate.
            wlhs = xpool.tile([P, 1], fp8, tag="wlhs")
            wrhs = xpool.tile([P, 256], fp8, tag="wrhs")
            nc.vector.memset(wlhs[:], 0.0)
            nc.vector.memset(wrhs[:], 0.0)
            warm_ps = psum_pool.tile([1, 256], f32, tag="warm")
            for _ in range(NWARM):
                nc.tensor.matmul(
                    warm_ps[:], wlhs[:], wrhs[:], start=True, stop=True,
                )

            # Four column groups (512 / 256 / 128 / 128), each with a
            # bank-aligned accumulation chain and reduce bank.  The stream
            # is k-major within each group, so earlier groups complete and
            # run their epilogues while later groups still stream; only
            # group 3's short [64, 128] chain sits in the tail.
            # Group 3 reuses group 0's PSUM banks (dead after group 0's
            # epilogue, ~57% into the stream, well before group 3's chain
            # starts at ~88%; the tile WAR deps enforce the ordering) --
            # 4 groups would otherwise need 9 banks and PSUM has 8.
            ax_abc = [
                psum_pool.tile([DIM, RH], f32, tag=f"axg{g}", name=f"axg{g}")
                for g in range(3)
            ]
            s_abc = [
                psum_pool.tile([DIM, RH], f32, tag=f"sg{g}", name=f"sg{g}")
                for g in range(3)
            ]
            ax_g = ax_abc + [ax_abc[0]]
            s_g = s_abc + [s_abc[0]]
            GCOLS = [
                slice(0, 512), slice(512, 768),
                slice(768, 896), slice(896, 1024),
            ]
            GW = [512, 256, 128, 128]

            # A-tile DMAs are all emitted upfront on the Sync queue; the
            # tile pool's buffer-reuse WAR dependencies gate issue depth.
            # Group 0 tiles: [p][kk8][512]; group 1: [p][q8][kk2][256];
            # groups 2/3: [p][q16][kk2][128] (DoubleRow pairs contiguous).
            # All tiles are 512KB with 4KB per-partition lines.
            a_tiles = []
            for idx in range(16):
                base = idx * P
                eng = nc.sync
                if idx < 8:
                    a_sb = apool.tile(
                        [P, KSUP, RH], fp8, tag="a0", name=f"a{idx}"
                    )
                    if idx == 0:
                        eng.dma_start(
                            out=a_sb[:, 0:4, :],
                            in_=a[base : base + P, 0 : 4 * RH],
                        )
                        eng.dma_start(
                            out=a_sb[:, 4:8, :],
                            in_=a[base : base + P, 4 * RH : 8 * RH],
                        )
                    else:
                        eng.dma_start(out=a_sb[:], in_=a[base : base + P, :])
                elif idx < 12:
                    a_sb = apool.tile(
                        [P, 8, 2, 256], fp8, tag="a1", name=f"a{idx}"
                    )
                    eng.dma_start(out=a_sb[:], in_=a[base : base + P, :])
                else:
                    a_sb = apool.tile(
                        [P, 16, 2, 128], fp8, tag="a23", name=f"a{idx}"
                    )
                    if idx == 15:
                        # Fine-grained last tile for the tail.
                        for sp in range(4):
                            eng.dma_start(
                                out=a_sb[:, 4 * sp : 4 * sp + 4, :, :],
                                in_=a[
                                    base : base + P,
                                    sp * 1024 : (sp + 1) * 1024,
                                ],
                            )
                    else:
                        eng.dma_start(out=a_sb[:], in_=a[base : base + P, :])
                a_tiles.append(a_sb)

            def acc_tile(g, st):
                """Accumulation matmuls for group g's super-tile st."""
                if g == 0:
                    a_sb = a_tiles[st]
                    for q in range(KSUP // 2):
                        t = st * (KSUP // 2) + q
                        nc.tensor.matmul(
                            ax_g[0][:],
                            x_sb[:, 2 * t : 2 * t + 2, :],
                            a_sb[:, 2 * q : 2 * q + 2, :],
                            start=(st == 0 and q == 0),
                            stop=(st == NSUP - 1 and q == KSUP // 2 - 1),
                            perf_mode=DR,
                        )
                elif g == 1:
                    a_sb = a_tiles[8 + st]
                    for q in range(8):
                        t = st * 8 + q
                        nc.tensor.matmul(
                            ax_g[1][:, 0:256],
                            x_sb[:, 2 * t : 2 * t + 2, :],
                            a_sb[:, q, :, :],
                            start=(st == 0 and q == 0),
                            stop=(st == 3 and q == 7),
                            perf_mode=DR,
                        )
                else:
                    a_sb = a_tiles[12 + (g - 2) * 2 + st]
                    for q in range(16):
                        t = st * 16 + q
                        nc.tensor.matmul(
                            ax_g[g][:, 0:128],
                            x_sb[:, 2 * t : 2 * t + 2, :],
                            a_sb[:, q, :, :],
                            start=(st == 0 and q == 0),
                            stop=(st == 1 and q == 15),
                            perf_mode=DR,
                        )

            def reduce_start(g):
                # Early part of the reduce: s_g = -B*xt + F, off the tail.
                nc.tensor.matmul(
                    s_g[g][:, 0 : GW[g]], w2[:], xv[:, GCOLS[g]],
                    start=True, stop=False,
                )

            def epilogue_stt(g):
                nc.vector.scalar_tensor_tensor(
                    tmp[:, GCOLS[g]], xv[0:DIM, GCOLS[g]], R_CONST,
                    ax_g[g][:, 0 : GW[g]],
                    op0=mult, op1=mult,
                )

            def epilogue_finish(g, store_eng):
                nc.tensor.matmul(
                    s_g[g][:, 0 : GW[g]], w1[:], tmp[:, GCOLS[g]],
                    start=False, stop=True,
                )
                nc.scalar.activation(
                    o_sb[:, GCOLS[g]], s_g[g][:, 0 : GW[g]], Copy
                )
                store_eng.dma_start(
                    out=out[:, GCOLS[g]], in_=o_sb[:, GCOLS[g]]
                )

            # ---- Group 0 (cols 0:512): 8 super-tiles.
            for st in range(NSUP):
                acc_tile(0, st)
                if st == 2:
                    for g in range(3):
                        reduce_start(g)
            epilogue_stt(0)

            # ---- Group 1 (cols 512:768): 4 super-tiles; each group's
            # reduce matmul is deferred past the next group's first
            # super-tile so the in-order PE queue never stalls on the DVE.
            for st in range(4):
                acc_tile(1, st)
                if st == 0:
                    epilogue_finish(0, nc.scalar)
            epilogue_stt(1)

            # ---- Group 2 (cols 768:896): 2 super-tiles.  Group 3's
            # reduce-start is emitted here, after group 0's s-bank is dead.
            for st in range(2):
                acc_tile(2, st)
                if st == 0:
                    reduce_start(3)
                    epilogue_finish(1, nc.scalar)
            epilogue_stt(2)

            # ---- Group 3 (cols 896:1024): 2 super-tiles.
            for st in range(2):
                acc_tile(3, st)
                if st == 0:
                    epilogue_finish(2, nc.scalar)

            # ---- Group 3 epilogue: the only chain in the tail, one
            # [64, 128] stt -> reduce matmul -> copy -> Sync-queue store.
            epilogue_stt(3)
            epilogue_finish(3, nc.sync)

    nc.finalize()
    return nc


def _get_nc():
    if "nc" not in _CACHE:
        _CACHE["nc"] = _build_nc()
    return _CACHE["nc"]


def _make_in_maps(x, A):
    import ml_dtypes

    e4 = ml_dtypes.float8_e4m3
    bf = ml_dtypes.bfloat16
    x = np.ascontiguousarray(np.asarray(x, dtype=np.float32))
    A = np.asarray(A, dtype=np.float32)

    x8 = x.astype(e4)
    # xp[p, kt, d] = x[kt*128 + p, d]
    xp = np.ascontiguousarray(x8.reshape(KT, P, DIM).transpose(1, 0, 2))
    A8 = A.astype(e4)

    w2 = np.zeros((DIM + 1, DIM), dtype=np.float32)
    w2[np.arange(DIM), np.arange(DIM)] = -B_CONST
    w2[DIM, :] = F_CONST
    w2 = w2.astype(bf)

    in_maps = []
    for c in range(NCORES):
        rows = slice(c * ROWS, (c + 1) * ROWS)
        ATc = A8[rows].T  # [8192 j, 1024 i]
        # Group 0 (cols 0:512): [st, kk, p, i] -> [st, p, kk, i].
        g0 = np.ascontiguousarray(
            ATc[:, 0:RH].reshape(NSUP, KSUP, P, RH).transpose(0, 2, 1, 3)
        ).reshape(NSUP * P, KSUP * RH)
        # Group 1 (cols 512:768): [st, q, kk, p, i] -> [st, p, q, kk, i]
        # (16 K-tiles per super-tile, DoubleRow pairs contiguous).
        g1 = np.ascontiguousarray(
            ATc[:, RH : RH + 256]
            .reshape(4, 8, 2, P, 256).transpose(0, 3, 1, 2, 4)
        ).reshape(4 * P, KSUP * RH)
        # Groups 2/3 (128 cols each): 32 K-tiles per super-tile.
        g23 = []
        for g in range(2):
            Ag = ATc[:, 768 + g * 128 : 768 + (g + 1) * 128]
            g23.append(
                np.ascontiguousarray(
                    Ag.reshape(2, 16, 2, P, 128).transpose(0, 3, 1, 2, 4)
                ).reshape(2 * P, KSUP * RH)
            )
        at = np.concatenate([g0, g1] + g23)
        in_maps.append(
            {
                "a": at,
                "xp": xp,
                "xt": np.ascontiguousarray(x[rows].T).astype(bf),
                "w2": w2,
            }
        )
    return in_maps


def run_sharded(x, A, trace=False, **kwargs):
    """Run the SPMD bass kernel; returns (full_output, BassKernelResults)."""
    from concourse.bass_utils import run_bass_kernel_spmd

    nc = _get_nc()
    res = run_bass_kernel_spmd(
        nc, _make_in_maps(x, A), core_ids=list(range(NCORES)), trace=trace, **kwargs
    )
    full = np.concatenate(
        [
            np.ascontiguousarray(res.results[c]["out"].astype(np.float32).T)
            for c in range(NCORES)
        ],
        axis=0,
    )
    return full.astype(np.float32, copy=False), res


def kernel(t, x, A):
    out, _ = run_sharded(x, A)
    return out
